# revision 17
# baseline (speedup 1.0000x reference)
import numpy as np

B = 8
SEQ = 4096
D = 1024
N_BASE = 10000.0
N_CORES = 8
SPC = SEQ // N_CORES   # seq rows per core (512)
H = 128                # f32 per 512B unit
UPP = 32               # units per partition per chunk (16KB)
UPC = SPC * D // H     # units per core chunk (4096)

_CACHE = {}


def _compute_pe() -> np.ndarray:
    """Mirror of the reference _pos_encoding (default jax backend, f32)."""
    import jax
    import jax.numpy as jnp

    pos = jnp.arange(SEQ, dtype=jnp.float32)[:, None]
    i = jnp.arange(D // 2, dtype=jnp.float32)
    denom = jnp.power(jnp.float32(N_BASE), 2.0 * i / jnp.float32(D))
    ang = pos / denom
    pe = jnp.stack([jnp.sin(ang), jnp.cos(ang)], axis=-1).reshape(SEQ, D)
    return np.asarray(jax.device_get(pe), dtype=np.float32)


def _pass_dmas(nc, engine, dram, row0, sbuf, u0, nu, to_sbuf, skip15):
    """Move [128 parts x nu units] between the chunk at dram[row0:]
    (natural order: partition p holds units [UPP*p, UPP*p+UPP)) and the
    SBUF region `sbuf` (a tile slice of shape [128, nu, H]), for unit
    columns [u0, u0+nu) of the chunk.

    skip15=False: one [128]-DMA -> 16 engines, nu/4 descs each.
    skip15=True: a [120]-DMA (engines 0-14) + an [8]-DMA (engines 0-7),
    so SDMA engine 15 (~17% slower than 0-14) gets nothing. HWDGE splits
    a DMA's n descriptors into runs of g = smallest divisor of n that is
    >= n/16, assigned to engines 0..n/g-1.
    """
    view = dram[row0 : row0 + 128 * UPP, :].rearrange(
        "(p j) d -> p j d", j=UPP
    )
    us = slice(u0, u0 + nu)
    if not skip15:
        pairs = [(sbuf[:, :, :], view[:, us, :])]
    else:
        pairs = [
            (sbuf[0:120, :, :], view[0:120, us, :]),
            (sbuf[120:128, :, :], view[120:128, us, :]),
        ]
    for sb, dr in pairs:
        if to_sbuf:
            engine.dma_start(out=sb, in_=dr)
        else:
            engine.dma_start(out=dr, in_=sb)


# sub-pass column plans, (u0, nu): chunk 0 ramps in small so the first
# add/write starts early; chunk 7 ramps out small so the final add
# barely delays the last writes. Others move as one full pass.
_PLANS = {
    0: [(0, 8), (8, 8), (16, 16)],
    B - 1: [(0, 16), (16, 8), (24, 8)],
}
_FULL = [(0, 32)]


def _build_program():
    import concourse.bacc as bacc
    import concourse.mybir as mybir
    import concourse.tile as tile

    nc = bacc.Bacc("TRN2")
    f32 = mybir.dt.float32
    x_in = nc.declare_dram_parameter("x", [B * UPC, H], f32, isOutput=False)
    pe_in = nc.declare_dram_parameter("pe", [UPC, H], f32, isOutput=False)
    y_out = nc.declare_dram_parameter("y", [B * UPC, H], f32, isOutput=True)

    with tile.TileContext(nc) as tc:
        with (
            tc.tile_pool(name="pe_pool", bufs=2) as pe_pool,
            tc.tile_pool(name="x_pool", bufs=B - 2) as x_pool,
            tc.tile_pool(name="sub_pool", bufs=6) as sub_pool,
        ):
            # pe halves in separate tiles (independent deps), one per
            # ring, both engine-15-free; peA gates the very first add.
            pe_a = pe_pool.tile([128, 16, H], f32)
            pe_b = pe_pool.tile([128, 16, H], f32)
            _pass_dmas(nc, nc.sync, pe_in, 0, pe_a, 0, 16, True, True)
            _pass_dmas(nc, nc.scalar, pe_in, 0, pe_b, 16, 16, True, True)

            def half_adds(xt, u0, nu):
                # an add operand can only span one pe tile; split the
                # column range at 16 where it crosses the pe_a/pe_b seam
                o = 0
                while nu > 0:
                    t, po = (pe_a, u0) if u0 < 16 else (pe_b, u0 - 16)
                    n = min(nu, 16 - (po % 16)) if u0 < 16 else nu
                    nc.vector.tensor_add(
                        xt[:, o : o + n, :],
                        xt[:, o : o + n, :],
                        t[:, po : po + n, :],
                    )
                    u0 += n
                    o += n
                    nu -= n

            # x sub-passes, each into its own tile for exact deps
            xts = {}
            for b in range(B):
                for u0, nu in _PLANS.get(b, _FULL):
                    pool = x_pool if nu == UPP else sub_pool
                    xt = pool.tile([128, nu, H], f32)
                    _pass_dmas(
                        nc, nc.sync, x_in, b * UPC, xt, u0, nu, True, False
                    )
                    xts[(b, u0)] = xt
            for b in range(B):
                for i, (u0, nu) in enumerate(_PLANS.get(b, _FULL)):
                    xt = xts[(b, u0)]
                    half_adds(xt, u0, nu)
                    # y7's first half is engine-15-free: with pe it
                    # shaves engine 15 to ~86% of the per-engine load,
                    # matching its ~17% lower throughput.
                    ys15 = b == B - 1 and i == 0
                    _pass_dmas(
                        nc, nc.scalar, y_out, b * UPC, xt, u0, nu,
                        False, ys15,
                    )
    if not nc.is_finalized():
        nc.finalize()
    return nc


def _get_state():
    if "nc" not in _CACHE:
        _CACHE["nc"] = _build_program()
    if "pe" not in _CACHE:
        _CACHE["pe"] = _compute_pe()
    return _CACHE["nc"], _CACHE["pe"]


def _in_maps(x, pe):
    in_maps = []
    for c in range(N_CORES):
        xs = np.ascontiguousarray(x[:, c * SPC : (c + 1) * SPC, :]).reshape(
            B * UPC, H
        )
        pes = np.ascontiguousarray(pe[c * SPC : (c + 1) * SPC, :]).reshape(
            UPC, H
        )
        in_maps.append({"x": xs, "pe": pes})
    return in_maps


def kernel(x, seq_len=None, **_):
    from concourse.bass_utils import run_bass_kernel_spmd

    x = np.asarray(x, dtype=np.float32)
    assert x.shape == (B, SEQ, D)
    if seq_len is not None:
        assert int(np.asarray(seq_len)) == SEQ

    nc, pe = _get_state()
    res = run_bass_kernel_spmd(nc, _in_maps(x, pe), list(range(N_CORES))).results

    out = np.empty((B, SEQ, D), dtype=np.float32)
    for c in range(N_CORES):
        out[:, c * SPC : (c + 1) * SPC, :] = res[c]["y"].reshape(B, SPC, D)
    return out
